# revision 36
# baseline (speedup 1.0000x reference)
"""Single-head causal attention (B=8, T=2048, D=1024, H=128) on 8 TRN2
NeuronCores — data-parallel over batch (one batch element per core).

Per-core dataflow (bf16 matmul compute, f32 accumulation):
  1. x [T, D] f32 loaded EAGERLY into a persistent SBUF buffer via 16
     tile DMAs issued all up front (evens on the sync HW queue, odds on
     gpsimd) so the DMA engines stream continuously and the scalar
     queue stays free for activations.
  2. Per tile, 8 TensorE transposes (bf16 truncation view of the f32
     data) build xT [d-part, dt, t]; PSUM->SBUF copies on DVE.
  3. Projections per 512-column chunk: qT/kT/vT [H, T] = W^T @ xT with
     weights stationary (8 accumulation matmuls per 512 cols, one PSUM
     bank wide). v natural tiles for PV come from PE-transposing vT
     tiles into v_aug [t-part, tt, H+1] whose last column stays 1.0
     (the ones column makes PV also produce the softmax denominator).
  4. Attention is interleaved per chunk right after its projections:
     scores TRANSPOSED per k-tile PAIR into a 2-bank PSUM tile
     ST[k 128, 2, q<=512] = kT_tile^T @ qT_chunk; ONE exp(scale*ST) on
     ScalarE per pair writes PT bf16 (already the lhsT orientation PV
     needs); diagonal tiles zero the 128x128 triangle with a DVE
     multiply against a precomputed causal mask.
  5. O[q 128, H+1] += PT_slice^T @ v_aug_tile accumulated over k tiles
     in a 4-bank PSUM accumulator; col H is the denominator. Divide on
     DVE into a per-chunk staging tile, ONE output DMA per chunk.
"""

import numpy as np

import concourse.bass as bass
import concourse.bacc as bacc
import concourse.mybir as mybir
import concourse.tile as tile
from concourse import bass_utils
from concourse.masks import make_identity

B, T, D, H = 8, 2048, 1024, 128
P = 128
DT = D // P  # 8 d tiles
TT = T // P  # 16 t tiles
CH = 512  # q chunk width
QC = T // CH  # 4 q chunks
N_CORES = 8
SCALE = float(1.0 / np.sqrt(H))
N_WARMUP = 16

F32 = mybir.dt.float32
BF16 = mybir.dt.bfloat16


def build_nc():
    nc = bacc.Bacc("TRN2", target_bir_lowering=False, debug=False)
    x = nc.dram_tensor("x", [T, D], F32, kind="ExternalInput").ap()
    wq_d = nc.dram_tensor("wq", [D, H], F32, kind="ExternalInput").ap()
    wk_d = nc.dram_tensor("wk", [D, H], F32, kind="ExternalInput").ap()
    wv_d = nc.dram_tensor("wv", [D, H], F32, kind="ExternalInput").ap()
    out = nc.dram_tensor("out", [T, H], F32, kind="ExternalOutput").ap()

    with tile.TileContext(nc) as tc:
        _build_body(nc, tc, x, wq_d, wk_d, wv_d, out)
    nc.compile()
    return nc


def _build_body(nc, tc, x, wq_d, wk_d, wv_d, out):
    with (
        tc.tile_pool(name="persist", bufs=1) as persist,
        tc.tile_pool(name="work", bufs=3) as work,
        tc.tile_pool(name="ps", bufs=1, space="PSUM") as ps,
    ):
        # ---- constants ----
        ident_b = persist.tile([P, P], BF16, tag="ident_b", name="ident_b")
        make_identity(nc, ident_b)

        # causal mask for diagonal 128x128 blocks of PT [k-part, q-col]:
        # keep q >= k (col >= partition); applied per-tile on DVE.
        mask = persist.tile([P, P], BF16, tag="mask", name="mask")
        nc.vector.memset(mask[:], 1.0)
        nc.gpsimd.affine_select(
            out=mask[:],
            in_=mask[:],
            compare_op=mybir.AluOpType.is_ge,
            fill=0.0,
            base=0,
            pattern=[[1, P]],
            channel_multiplier=-1,
        )

        # ---- persistent activations ----
        # one tile per 128-row slab of x so transposes depend only on
        # their own slab's DMA (a single big tile would serialize the
        # first transpose behind the whole x load)
        x_nat = [
            persist.tile([P, D], F32, tag=f"x_nat{tt}", name=f"x_nat{tt}")
            for tt in range(TT)
        ]
        xT = persist.tile([P, DT, T], BF16, tag="xT", name="xT")
        qT = persist.tile([P, T], BF16, tag="qT", name="qT")
        kT = persist.tile([P, T], BF16, tag="kT", name="kT")
        vT = persist.tile([P, T], BF16, tag="vT", name="vT")
        v_aug = persist.tile([P, TT, H + 1], BF16, tag="v_aug", name="v_aug")
        # only the ones column (col H) needs init; cols 0:H get overwritten
        nc.gpsimd.memset(v_aug[:, :, H : H + 1], 1.0)
        # dedicated warmup operand (never aliased with real data)
        warm_src = persist.tile([P, CH], BF16, tag="warm_src", name="warm_src")
        nc.vector.memset(warm_src[:], 1.0)

        # ---- eager DMA, ALL on the sync HWDGE ring in dependency order:
        # a second ring starves under load, so ordering on one ring is the
        # only way to control arrival times. x0-3 unlock the first
        # transposes, then weights (needed by chunk-0 projections), then
        # the rest of x. ----
        w_stage = {}
        for tt in range(2):
            nc.sync.dma_start(x_nat[tt][:], x[tt * P : (tt + 1) * P, :])
        for nm, wd in (("wq", wq_d), ("wk", wk_d), ("wv", wv_d)):
            wf = work.tile([P, DT, H], F32, tag="wf32", name=f"{nm}_f32")
            nc.sync.dma_start(wf[:], wd.rearrange("(a p) h -> p a h", p=P))
            w_stage[nm] = wf
        for tt in range(2, TT):
            nc.sync.dma_start(x_nat[tt][:], x[tt * P : (tt + 1) * P, :])

        # cast weights to bf16 (DVE)
        w_bf = []
        for nm in ("wq", "wk", "wv"):
            wb = persist.tile([P, DT, H], BF16, tag=f"{nm}_bf", name=f"{nm}_bf")
            nc.vector.tensor_copy(wb[:], w_stage[nm][:])
            w_bf.append(wb)
        wq_bf, wk_bf, wv_bf = w_bf

        # ---- PE warmup during the DMA window (HAM clock ramp) ----
        warm_n = [0]

        def emit_warm(k):
            # transpose-mode matmuls do NOT count as PE-busy for HAM, so
            # sprinkle real matmuls through transpose-only stretches or the
            # clock re-throttles to 1.2 GHz
            for _ in range(k):
                warm_ps = ps.tile(
                    [P, CH], F32, tag="o", bufs=4, name=f"warm{warm_n[0]}"
                )
                warm_n[0] += 1
                nc.tensor.matmul(
                    warm_ps[:], ident_b[:], warm_src[:], start=True, stop=True
                )

        emit_warm(N_WARMUP)

        # ---- main loop: per 512-col chunk, transposes + projections then
        # attention for that chunk ----
        for c in range(QC):
            t0 = c * CH
            # transposes: 8 per t-tile via bf16-truncation view of x_nat
            for tt in range(4 * c, 4 * c + 4):
                xv = x_nat[tt].bitcast(BF16)  # [P, 2*D]
                for half in range(2):
                    tr_ps = ps.tile(
                        [P, 4 * P], BF16, tag="st", bufs=4, name=f"tr{tt}_{half}"
                    )
                    for j in range(4):
                        dt = half * 4 + j
                        nc.tensor.transpose(
                            tr_ps[:, j * P : (j + 1) * P],
                            xv[:, 2 * dt * P + 1 : 2 * (dt + 1) * P : 2],
                            ident_b,
                        )
                    dst = xT[:, half * 4 : half * 4 + 4, tt * P : (tt + 1) * P]
                    src = tr_ps.rearrange("p (a t) -> p a t", a=4)
                    nc.vector.tensor_copy(dst, src)

            # projections (weights stationary, 8 accumulation steps, 512 wide)
            for nm, wb, dstT in (
                ("q", wq_bf, qT),
                ("k", wk_bf, kT),
                ("v", wv_bf, vT),
            ):
                pr_ps = ps.tile([P, CH], F32, tag="o", bufs=4, name=f"{nm}T_ps{c}")
                for dt in range(DT):
                    nc.tensor.matmul(
                        pr_ps[:],
                        wb[:, dt, :],
                        xT[:, dt, t0 : t0 + CH],
                        start=(dt == 0),
                        stop=(dt == DT - 1),
                    )
                if nm == "v":
                    # v path is off the scores' critical path: copy on
                    # ScalarE, which idles at chunk boundaries
                    nc.scalar.copy(dstT[:, t0 : t0 + CH], pr_ps[:])
                else:
                    nc.vector.tensor_copy(dstT[:, t0 : t0 + CH], pr_ps[:])
            # v natural tiles for this chunk: PE-transpose vT tiles into v_aug
            for tt in range(4 * c, 4 * c + 4):
                vtr = ps.tile([P, P], BF16, tag="st", bufs=4, name=f"vtr{tt}")
                nc.tensor.transpose(vtr[:], vT[:, tt * P : (tt + 1) * P], ident_b)
                nc.scalar.copy(v_aug[:, tt, 0:H], vtr[:])

            # ---- attention for this chunk (k tiles 0 .. 4c+3) ----
            n_k = 4 * c + 4
            o_ps = [
                ps.tile([P, H + 1], F32, tag="o", bufs=4, name=f"o{c}_{s}")
                for s in range(4)
            ]
            st_tiles = {}

            def emit_score(i, c=c, t0=t0, st_tiles=st_tiles):
                e0 = max(i - 4 * c, 0) * P
                st = ps.tile([P, CH], F32, tag="st", bufs=4, name=f"st{c}_{i}")
                nc.tensor.matmul(
                    st[:, e0:],
                    kT[:, i * P : (i + 1) * P],
                    qT[:, t0 + e0 : t0 + CH],
                    start=True,
                    stop=True,
                )
                st_tiles[i] = st

            emit_score(0)
            for i in range(n_k):
                if i + 1 < n_k:
                    emit_score(i + 1)  # keep PE fed while ACT does exp(i)
                st = st_tiles.pop(i)
                e0 = max(i - 4 * c, 0) * P
                pt = work.tile([P, CH], BF16, tag="pt", bufs=4, name=f"pt{c}_{i}")
                nc.scalar.activation(
                    pt[:, e0:],
                    st[:, e0:],
                    mybir.ActivationFunctionType.Exp,
                    scale=SCALE,
                )
                if i >= 4 * c:
                    # zero the causal triangle of the diagonal block (DVE)
                    nc.vector.tensor_mul(
                        pt[:, e0 : e0 + P], pt[:, e0 : e0 + P], mask[:]
                    )
                for s in range(4):
                    if i <= 4 * c + s:
                        nc.tensor.matmul(
                            o_ps[s][:],
                            pt[:, s * P : (s + 1) * P],
                            v_aug[:, i, :],
                            start=(i == 0),
                            stop=(i == 4 * c + s),
                        )
            o_sb = work.tile([P, 4, H], F32, tag="o_sb", bufs=2, name=f"o_sb{c}")
            for s in range(4):
                qt_idx = 4 * c + s
                recip = work.tile([P, 1], F32, tag="recip", name=f"rcp{qt_idx}")
                nc.vector.reciprocal(recip[:], o_ps[s][:, H : H + 1])
                nc.vector.tensor_scalar_mul(
                    o_sb[:, s, :], o_ps[s][:, 0:H], recip[:]
                )
                if c == QC - 1:
                    # last chunk: per-tile stores so s=0..2 fly while the
                    # exp chain still runs; only a 128x128 DMA trails s=3
                    nc.sync.dma_start(
                        out[qt_idx * P : (qt_idx + 1) * P, :], o_sb[:, s, :]
                    )
            if c < QC - 1:
                nc.sync.dma_start(
                    out[t0 : t0 + CH, :].rearrange("(a p) h -> p a h", p=P),
                    o_sb[:],
                )


_NC_CACHE = None


def _get_nc():
    global _NC_CACHE
    if _NC_CACHE is None:
        _NC_CACHE = build_nc()
    return _NC_CACHE


def kernel(**inputs):
    x = np.ascontiguousarray(np.asarray(inputs["x"], dtype=np.float32))
    wq = np.ascontiguousarray(np.asarray(inputs["Wq"], dtype=np.float32))
    wk = np.ascontiguousarray(np.asarray(inputs["Wk"], dtype=np.float32))
    wv = np.ascontiguousarray(np.asarray(inputs["Wv"], dtype=np.float32))
    assert x.shape == (B, T, D)
    nc = _get_nc()
    in_maps = [
        {"x": np.ascontiguousarray(x[b]), "wq": wq, "wk": wk, "wv": wv}
        for b in range(N_CORES)
    ]
    res = bass_utils.run_bass_kernel_spmd(nc, in_maps, core_ids=list(range(N_CORES)))
    return np.stack([res.results[b]["out"] for b in range(N_CORES)], axis=0)


# revision 38
# speedup vs baseline: 1.1469x; 1.1469x over previous
"""Single-head causal attention (B=8, T=2048, D=1024, H=128) on 8 TRN2
NeuronCores — data-parallel over batch (one batch element per core).

Per-core dataflow (bf16 matmul compute, f32 accumulation):
  1. All input DMAs issued up front on the SINGLE sync HWDGE ring in
     dependency order [x0, x1, wq, wk, wv, x2..x15] — splitting across
     the two HW rings starves the second ring under load, and the
     ordering controls arrival times exactly.  16 warmup matmuls ramp
     the HAM clock gate while DMA streams (transpose-mode matmuls do
     NOT count as PE-busy for HAM, so warmups must bridge until the
     first projection matmuls).
  2. Per x tile, 8 TensorE transposes (bf16 truncation view of the f32
     data) build xT [d-part, dt, t]; PSUM->SBUF copies on DVE.
  3. Projections per 512-column chunk: qT/kT/vT [H, T] = W^T @ xT with
     weights stationary (8 accumulation matmuls per 512 cols, one PSUM
     bank wide). q/k copies on DVE; the v-path copies (vT and the PE
     re-transposed v_aug natural tiles) run on ScalarE, which idles at
     chunk boundaries. v_aug [t-part, tt, H+1] keeps column H at 1.0
     (the ones column makes PV also produce the softmax denominator).
  4. Attention per chunk: scores TRANSPOSED per k-tile ST[k 128,
     q<=512] = kT_tile^T @ qT_chunk into one PSUM bank; exp(scale*ST)
     on ScalarE writes PT bf16 (already the lhsT orientation PV
     needs); diagonal tiles exp only the valid column range and zero
     the 128x128 triangle with a DVE multiply against a causal mask.
  5. O[q 128, H+1] += PT_slice^T @ v_aug_tile accumulated over k tiles
     (one PSUM bank per q-subtile — a start=True matmul clears its
     whole bank, so accumulation groups must not share banks); col H
     is the denominator. Divide on DVE into a per-chunk staging tile,
     ONE output DMA per chunk on sync.
"""

import numpy as np

import concourse.bass as bass
import concourse.bacc as bacc
import concourse.mybir as mybir
import concourse.tile as tile
from concourse import bass_utils
from concourse.masks import make_identity

B, T, D, H = 8, 2048, 1024, 128
P = 128
DT = D // P  # 8 d tiles
TT = T // P  # 16 t tiles
CH = 512  # q chunk width
QC = T // CH  # 4 q chunks
N_CORES = 8
SCALE = float(1.0 / np.sqrt(H))
N_WARMUP = 16

F32 = mybir.dt.float32
BF16 = mybir.dt.bfloat16


def build_nc():
    nc = bacc.Bacc("TRN2", target_bir_lowering=False, debug=False)
    x = nc.dram_tensor("x", [T, D], F32, kind="ExternalInput").ap()
    wq_d = nc.dram_tensor("wq", [D, H], F32, kind="ExternalInput").ap()
    wk_d = nc.dram_tensor("wk", [D, H], F32, kind="ExternalInput").ap()
    wv_d = nc.dram_tensor("wv", [D, H], F32, kind="ExternalInput").ap()
    out = nc.dram_tensor("out", [T, H], F32, kind="ExternalOutput").ap()

    with tile.TileContext(nc) as tc:
        _build_body(nc, tc, x, wq_d, wk_d, wv_d, out)
    nc.compile()
    return nc


def _build_body(nc, tc, x, wq_d, wk_d, wv_d, out):
    with (
        tc.tile_pool(name="persist", bufs=1) as persist,
        tc.tile_pool(name="work", bufs=3) as work,
        tc.tile_pool(name="ps", bufs=1, space="PSUM") as ps,
    ):
        # ---- constants ----
        ident_b = persist.tile([P, P], BF16, tag="ident_b", name="ident_b")
        make_identity(nc, ident_b)

        # causal mask for diagonal 128x128 blocks of PT [k-part, q-col]:
        # keep q >= k (col >= partition); applied per-tile on DVE.
        mask = persist.tile([P, P], BF16, tag="mask", name="mask")
        nc.vector.memset(mask[:], 1.0)
        nc.gpsimd.affine_select(
            out=mask[:],
            in_=mask[:],
            compare_op=mybir.AluOpType.is_ge,
            fill=0.0,
            base=0,
            pattern=[[1, P]],
            channel_multiplier=-1,
        )

        # ---- persistent activations ----
        # one tile per 128-row slab of x so transposes depend only on
        # their own slab's DMA (a single big tile would serialize the
        # first transpose behind the whole x load)
        x_nat = [
            persist.tile([P, D], F32, tag=f"x_nat{tt}", name=f"x_nat{tt}")
            for tt in range(TT)
        ]
        xT = persist.tile([P, DT, T], BF16, tag="xT", name="xT")
        qT = persist.tile([P, T], BF16, tag="qT", name="qT")
        kT = persist.tile([P, T], BF16, tag="kT", name="kT")
        vT = persist.tile([P, T], BF16, tag="vT", name="vT")
        v_aug = persist.tile([P, TT, H + 1], BF16, tag="v_aug", name="v_aug")
        # only the ones column (col H) needs init; cols 0:H get overwritten
        nc.gpsimd.memset(v_aug[:, :, H : H + 1], 1.0)
        # dedicated warmup operand (never aliased with real data)
        warm_src = persist.tile([P, CH], BF16, tag="warm_src", name="warm_src")
        nc.vector.memset(warm_src[:], 1.0)

        # ---- eager DMA, ALL on the sync HWDGE ring in dependency order:
        # a second ring starves under load, so ordering on one ring is the
        # only way to control arrival times. x0-3 unlock the first
        # transposes, then weights (needed by chunk-0 projections), then
        # the rest of x. ----
        w_stage = {}
        for tt in range(2):
            nc.sync.dma_start(x_nat[tt][:], x[tt * P : (tt + 1) * P, :])
        for nm, wd in (("wq", wq_d), ("wk", wk_d), ("wv", wv_d)):
            wf = work.tile([P, DT, H], F32, tag="wf32", name=f"{nm}_f32")
            nc.sync.dma_start(wf[:], wd.rearrange("(a p) h -> p a h", p=P))
            w_stage[nm] = wf
        for tt in range(2, TT):
            nc.sync.dma_start(x_nat[tt][:], x[tt * P : (tt + 1) * P, :])

        # cast weights to bf16 (DVE)
        w_bf = []
        for nm in ("wq", "wk", "wv"):
            wb = persist.tile([P, DT, H], BF16, tag=f"{nm}_bf", name=f"{nm}_bf")
            nc.vector.tensor_copy(wb[:], w_stage[nm][:])
            w_bf.append(wb)
        wq_bf, wk_bf, wv_bf = w_bf

        # ---- PE warmup during the DMA window (HAM clock ramp) ----
        warm_n = [0]

        def emit_warm(k):
            # transpose-mode matmuls do NOT count as PE-busy for HAM, so
            # sprinkle real matmuls through transpose-only stretches or the
            # clock re-throttles to 1.2 GHz
            for _ in range(k):
                warm_ps = ps.tile(
                    [P, CH], F32, tag="o", bufs=4, name=f"warm{warm_n[0]}"
                )
                warm_n[0] += 1
                nc.tensor.matmul(
                    warm_ps[:], ident_b[:], warm_src[:], start=True, stop=True
                )

        emit_warm(N_WARMUP)

        # ---- main loop: per 512-col chunk, transposes + projections then
        # attention for that chunk ----
        for c in range(QC):
            t0 = c * CH
            # transposes: 8 per t-tile via bf16-truncation view of x_nat
            for tt in range(4 * c, 4 * c + 4):
                xv = x_nat[tt].bitcast(BF16)  # [P, 2*D]
                for half in range(2):
                    tr_ps = ps.tile(
                        [P, 4 * P], BF16, tag="st", bufs=4, name=f"tr{tt}_{half}"
                    )
                    for j in range(4):
                        dt = half * 4 + j
                        nc.tensor.transpose(
                            tr_ps[:, j * P : (j + 1) * P],
                            xv[:, 2 * dt * P + 1 : 2 * (dt + 1) * P : 2],
                            ident_b,
                        )
                    dst = xT[:, half * 4 : half * 4 + 4, tt * P : (tt + 1) * P]
                    src = tr_ps.rearrange("p (a t) -> p a t", a=4)
                    nc.vector.tensor_copy(dst, src)

            # projections (weights stationary, 8 accumulation steps, 512 wide)
            for nm, wb, dstT in (
                ("q", wq_bf, qT),
                ("k", wk_bf, kT),
                ("v", wv_bf, vT),
            ):
                pr_ps = ps.tile([P, CH], F32, tag="o", bufs=4, name=f"{nm}T_ps{c}")
                for dt in range(DT):
                    nc.tensor.matmul(
                        pr_ps[:],
                        wb[:, dt, :],
                        xT[:, dt, t0 : t0 + CH],
                        start=(dt == 0),
                        stop=(dt == DT - 1),
                    )
                if nm == "v":
                    # v path is off the scores' critical path: copy on
                    # ScalarE, which idles at chunk boundaries
                    nc.scalar.copy(dstT[:, t0 : t0 + CH], pr_ps[:])
                else:
                    nc.vector.tensor_copy(dstT[:, t0 : t0 + CH], pr_ps[:])
            # v natural tiles for this chunk: PE-transpose vT tiles into v_aug
            for tt in range(4 * c, 4 * c + 4):
                vtr = ps.tile([P, P], BF16, tag="st", bufs=4, name=f"vtr{tt}")
                nc.tensor.transpose(vtr[:], vT[:, tt * P : (tt + 1) * P], ident_b)
                nc.scalar.copy(v_aug[:, tt, 0:H], vtr[:])

            # ---- attention for this chunk (k tiles 0 .. 4c+3) ----
            n_k = 4 * c + 4
            o_ps = [
                ps.tile([P, H + 1], F32, tag="o", bufs=4, name=f"o{c}_{s}")
                for s in range(4)
            ]
            st_tiles = {}

            def emit_score(i, c=c, t0=t0, st_tiles=st_tiles):
                e0 = max(i - 4 * c, 0) * P
                st = ps.tile([P, CH], F32, tag="st", bufs=4, name=f"st{c}_{i}")
                nc.tensor.matmul(
                    st[:, e0:],
                    kT[:, i * P : (i + 1) * P],
                    qT[:, t0 + e0 : t0 + CH],
                    start=True,
                    stop=True,
                )
                st_tiles[i] = st

            emit_score(0)
            for i in range(n_k):
                if i + 1 < n_k:
                    emit_score(i + 1)  # keep PE fed while ACT does exp(i)
                st = st_tiles.pop(i)
                e0 = max(i - 4 * c, 0) * P
                pt = work.tile([P, CH], BF16, tag="pt", bufs=4, name=f"pt{c}_{i}")
                nc.scalar.activation(
                    pt[:, e0:],
                    st[:, e0:],
                    mybir.ActivationFunctionType.Exp,
                    scale=SCALE,
                )
                if i >= 4 * c:
                    # zero the causal triangle of the diagonal block (DVE)
                    nc.vector.tensor_mul(
                        pt[:, e0 : e0 + P], pt[:, e0 : e0 + P], mask[:]
                    )
                for s in range(4):
                    if i <= 4 * c + s:
                        nc.tensor.matmul(
                            o_ps[s][:],
                            pt[:, s * P : (s + 1) * P],
                            v_aug[:, i, :],
                            start=(i == 0),
                            stop=(i == 4 * c + s),
                        )
            o_sb = work.tile([P, 4, H], F32, tag="o_sb", bufs=2, name=f"o_sb{c}")
            for s in range(4):
                qt_idx = 4 * c + s
                recip = work.tile([P, 1], F32, tag="recip", name=f"rcp{qt_idx}")
                nc.vector.reciprocal(recip[:], o_ps[s][:, H : H + 1])
                nc.vector.tensor_scalar_mul(
                    o_sb[:, s, :], o_ps[s][:, 0:H], recip[:]
                )
            nc.sync.dma_start(
                out[t0 : t0 + CH, :].rearrange("(a p) h -> p a h", p=P),
                o_sb[:],
            )


_NC_CACHE = None


def _get_nc():
    global _NC_CACHE
    if _NC_CACHE is None:
        _NC_CACHE = build_nc()
    return _NC_CACHE


def kernel(**inputs):
    x = np.ascontiguousarray(np.asarray(inputs["x"], dtype=np.float32))
    wq = np.ascontiguousarray(np.asarray(inputs["Wq"], dtype=np.float32))
    wk = np.ascontiguousarray(np.asarray(inputs["Wk"], dtype=np.float32))
    wv = np.ascontiguousarray(np.asarray(inputs["Wv"], dtype=np.float32))
    assert x.shape == (B, T, D)
    nc = _get_nc()
    in_maps = [
        {"x": np.ascontiguousarray(x[b]), "wq": wq, "wk": wk, "wv": wv}
        for b in range(N_CORES)
    ]
    res = bass_utils.run_bass_kernel_spmd(nc, in_maps, core_ids=list(range(N_CORES)))
    return np.stack([res.results[b]["out"] for b in range(N_CORES)], axis=0)
